# revision 20
# baseline (speedup 1.0000x reference)
"""Trainium2 Bass kernel for nn_GAT_34059090657327 (6-layer GAT + JKN + attention pooling).

V4 on top of V3:
  - Gathers rotate across SWDGE queues 1-3 (num_swdge_queues=4): descriptor
    generation for queues >0 runs asynchronously on idle Q7 core pairs, tripling
    effective gather throughput (~25us vs ~76us per 8.5k-idx gather).
  - Layer 0 gathers from a 128-row dictionary table (emb@W0|a_s precomputed on
    host) using x_idx[src] indices: no AllGather(0), no x_init one-hot matmuls.
    nm0 staging is 20 one-hot matmuls against a [128,66] table.
  - Per-layer AllGather split in two (slices 0-2 after stage 2, slices 3-4 at
    layer end) so most of the exchange hides under the gather stream.
  - Per-group softmax chain moved to the ACT engine (Lrelu, Exp with accum_out
    denominator, normalize via PSUM-side Copy*scale) - the DVE was being
    port-starved by SWDGE descriptor traffic.
  - Gather batches sized descending so the last batch's consumers (which gate
    staging of the next layer) finish sooner.
  - Layer-5 JKN + pooling feature matmuls run inside the layer-5 MLP slices.
Distribution: dst-sharded nodes+edges, per-layer compact bf16 AllGather of the
node table (h|a_s), local expand to 256B rows, identity-matmul scatter per group.
"""
import numpy as np
import sys

sys.path.insert(0, '/opt/trn_rl_repo')

import concourse.bass as bass
import concourse.mybir as mybir
import concourse.tile as tile
from concourse import library_config
from concourse.bass import AP
from concourse.bass_utils import run_bass_kernel_spmd
from concourse.library_overlay import lower_extended_insts
from concourse.tile_rust import add_dep_helper

F32 = mybir.dt.float32
F32R = mybir.dt.float32r
BF16 = mybir.dt.bfloat16
I16 = mybir.dt.int16
OP = mybir.AluOpType
ACTF = mybir.ActivationFunctionType

N, E, NG, DIM, HID, L = 20000, 320000, 64, 128, 64, 6
NC = 8
NPC = N // NC            # 2500
P = 128
GRP = 20                 # node groups of 128 per core
NPAD = GRP * P           # 2560
LRELU = 0.2
NROW = 66                # compact node-table row: h(64) | a_s | a_d
NB = 5                   # gather batches per layer (GRP/NB groups each)
GQ = (1, 2, 3, 1, 2)     # SWDGE queue per gather batch (queues 1-3 run async)
AGS = 3                  # slices covered by the early AllGather piece

_cache = {}

# ---------------------------------------------------------------------------
# This walrus build encodes only ONE semaphore wait/update per TPB_CTRL
# instruction ("Too many sync wait commands" on the Tile tail drain). Split
# extra waits onto preceding NoOps at BIR-serialization time.
import json as _json


def _split_multiwaits(js: bytes) -> bytes:
    j = _json.loads(js)
    n = 0
    for fn in j["functions"]:
        for bb in fn["blocks"]:
            out = []
            for inst in bb["instructions"]:
                si = inst.get("sync_info") or {}
                waits = si.get("on_wait") or []
                if len(waits) > 1:
                    for w in waits[:-1]:
                        n += 1
                        out.append({
                            "name": inst["name"] + f"_w{n}", "opcode": "NoOp",
                            "engine": inst["engine"], "ins": [], "outs": [],
                            "sync_info": {"on_wait": [w], "on_update": []},
                        })
                    si["on_wait"] = [waits[-1]]
                out.append(inst)
                ups = si.get("on_update") or []
                if len(ups) > 1 and inst["opcode"] in ("NoOp", "Drain", "EventSemaphore"):
                    si["on_update"] = [ups[0]]
                    for u in ups[1:]:
                        n += 1
                        out.append({
                            "name": inst["name"] + f"_u{n}", "opcode": "NoOp",
                            "engine": inst["engine"], "ins": [], "outs": [],
                            "sync_info": {"on_wait": [], "on_update": [u]},
                        })
            bb["instructions"] = out
    return _json.dumps(j).encode()


if not getattr(bass.Bass, "_mw_patched", False):
    _orig_to_json_bytes = bass.Bass.to_json_bytes

    def _to_json_bytes_patched(self, *a, **k):
        return _split_multiwaits(_orig_to_json_bytes(self, *a, **k))

    bass.Bass.to_json_bytes = _to_json_bytes_patched
    bass.Bass._mw_patched = True



# ---------------------------------------------------------------------------
# Relax dma_gather's 256B elem-SIZE restriction for non-transpose gathers (the
# bass assert notes it is a "transpose restriction"; the element STRIDE stays
# 256B-aligned, which is the actual ISA field granularity). Installed as a
# textual patch of the original method so upstream changes surface loudly.
import inspect as _inspect

if not getattr(bass.BassGpSimd, "_gather_relaxed", False):
    _gsrc = _inspect.getsource(bass.BassGpSimd.dma_gather)
    _old_assert = (
        "assert (\n"
        "            elem_size_bytes > 0 and elem_size_bytes % 256 == 0\n"
        "        )  # transpose restriction"
    )
    assert _old_assert in _gsrc, "dma_gather source changed; re-derive patch"
    _gsrc = _gsrc.replace(
        _old_assert,
        "assert elem_size_bytes > 0 and (\n"
        "            elem_size_bytes % 256 == 0 or (not transpose and elem_size_bytes % 2 == 0)\n"
        "        )",
    )
    _gsrc = "def _dma_gather_relaxed" + _gsrc[_gsrc.index("("):]
    _ns = vars(bass).copy()
    exec(compile(_gsrc, "<dma_gather_relaxed>", "exec"), _ns)
    bass.BassGpSimd.dma_gather = _ns["_dma_gather_relaxed"]
    bass.BassGpSimd._gather_relaxed = True


def _bc(ap, pos, count):
    """Insert a stride-0 (broadcast) dim of `count` at free-dim position `pos`."""
    lst = [list(x) for x in ap.ap]
    lst.insert(1 + pos, [0, count])
    return AP(ap.tensor, ap.offset, lst)


def _f(ap):
    """View a float32r AP as plain fp32 for DVE/ACT consumers."""
    return ap.bitcast(F32)


def _build(Ms):
    Ms = list(Ms)
    assert len(Ms) == GRP
    off = np.zeros(GRP + 1, np.int64)
    np.cumsum(Ms, out=off[1:])
    C = int(off[GRP])         # total chunks per core
    GPB = GRP // NB           # groups per gather batch
    MMAX = max(Ms)
    CBMAX = max(int(off[(b + 1) * GPB] - off[b * GPB]) for b in range(NB))

    nc = bass.Bass(num_devices=NC, num_swdge_queues=4)

    # ---------------- inputs ----------------
    e_idxw = nc.dram_tensor("e_idxw", [P, C * 8], I16, kind="ExternalInput")
    e_idxw0 = nc.dram_tensor("e_idxw0", [P, C * 8], I16, kind="ExternalInput")
    e_gt0 = nc.dram_tensor("e_gt0", [P, P], BF16, kind="ExternalInput")
    e_ae = nc.dram_tensor("e_ae", [P, C, 6], BF16, kind="ExternalInput")
    e_aeloop = nc.dram_tensor("e_aeloop", [P, GRP, L], F32, kind="ExternalInput")
    e_spool = nc.dram_tensor("e_spool", [P, GRP, 8], F32, kind="ExternalInput")
    e_pcnt = nc.dram_tensor("e_pcnt", [8, 1], F32, kind="ExternalInput")
    e_xidx = nc.dram_tensor("e_xidx", [NPAD], F32, kind="ExternalInput")
    w_iotac = nc.dram_tensor("w_iotac", [P, 1], F32, kind="ExternalInput")
    w_ident = nc.dram_tensor("w_ident", [P, P], F32, kind="ExternalInput")
    w_t0 = nc.dram_tensor("w_t0", [P, NROW], F32, kind="ExternalInput")
    w_conv = nc.dram_tensor("w_conv", [P, L * HID], F32, kind="ExternalInput")
    w_att = nc.dram_tensor("w_att", [HID, L * 2], F32, kind="ExternalInput")
    w_m1 = nc.dram_tensor("w_m1", [HID, L * HID], F32, kind="ExternalInput")
    w_m2 = nc.dram_tensor("w_m2", [HID, L * HID], F32, kind="ExternalInput")
    w_m3 = nc.dram_tensor("w_m3", [HID, L * DIM], F32, kind="ExternalInput")
    w_b1 = nc.dram_tensor("w_b1", [HID, L], F32, kind="ExternalInput")
    w_b2 = nc.dram_tensor("w_b2", [HID, L], F32, kind="ExternalInput")
    w_b3 = nc.dram_tensor("w_b3", [DIM, L], F32, kind="ExternalInput")
    w_eps = nc.dram_tensor("w_eps", [P, L], F32, kind="ExternalInput")
    w_g1w1 = nc.dram_tensor("w_g1w1", [DIM, HID], F32, kind="ExternalInput")
    w_g1b1 = nc.dram_tensor("w_g1b1", [HID, 1], F32, kind="ExternalInput")
    w_g1w2 = nc.dram_tensor("w_g1w2", [HID, 1], F32, kind="ExternalInput")
    w_g1b2 = nc.dram_tensor("w_g1b2", [1, 1], F32, kind="ExternalInput")
    w_g2w1 = nc.dram_tensor("w_g2w1", [DIM, HID], F32, kind="ExternalInput")
    w_g2b1 = nc.dram_tensor("w_g2b1", [HID, 1], F32, kind="ExternalInput")
    w_g2w2 = nc.dram_tensor("w_g2w2", [HID, DIM], F32, kind="ExternalInput")
    w_g2b2 = nc.dram_tensor("w_g2b2", [DIM, 1], F32, kind="ExternalInput")
    w_pw1 = nc.dram_tensor("w_pw1", [DIM, HID], F32, kind="ExternalInput")
    w_pb1 = nc.dram_tensor("w_pb1", [HID, 1], F32, kind="ExternalInput")
    w_pw2 = nc.dram_tensor("w_pw2", [HID, HID], F32, kind="ExternalInput")
    w_pb2 = nc.dram_tensor("w_pb2", [HID, 1], F32, kind="ExternalInput")
    w_pw3 = nc.dram_tensor("w_pw3", [HID, 1], F32, kind="ExternalInput")
    w_pb3 = nc.dram_tensor("w_pb3", [1, 1], F32, kind="ExternalInput")
    eout = nc.dram_tensor("out", [1, 8], F32, kind="ExternalOutput")

    with tile.TileContext(nc) as tc:
        with tc.tile_pool(name="c1", bufs=1) as c1, \
             tc.tile_pool(name="gp", bufs=1) as gp, \
             tc.tile_pool(name="zp", bufs=2) as zp, \
             tc.tile_pool(name="sm", bufs=3) as sm, \
             tc.tile_pool(name="stg", bufs=2) as stgp, \
             tc.tile_pool(name="yp", bufs=2) as yp, \
             tc.tile_pool(name="dr", bufs=1, space="DRAM") as dr:
            ps_stack = tc.tile_pool(name="psA", bufs=4, space="PSUM")
            psA = ps_stack.__enter__()
            ps_stackB = tc.tile_pool(name="psB", bufs=3, space="PSUM")
            psB = ps_stackB.__enter__()
            ps_stackP = tc.tile_pool(name="psP", bufs=1, space="PSUM")
            psP = ps_stackP.__enter__()

            rel = nc.gpsimd.load_library(library_config.mlp)
            nig_regs = {}
            for b in range(NB):
                nig = int(off[(b + 1) * GPB] - off[b * GPB]) * P
                if nig not in nig_regs:
                    nig_regs[nig] = nc.gpsimd.to_reg(nig)

            def load(t, shape, tag, dtype=F32):
                s = c1.tile(shape, dtype, tag=tag)
                nc.sync.dma_start(out=s[:], in_=t[:])
                return s

            # --- layer-0 gather critical path: only the index grids + library
            idxw0 = load(e_idxw0, [P, C * 8], "idxw0", dtype=I16)

            # DRAM comm buffers for layers 1..L-1: compact bf16 AllGather of
            # (h|a_s) + expanded 256B-row gather table. Split: piece A covers
            # node rows [0, AGS*512), piece B the rest.
            NAG = 65
            RA = AGS * 512              # rows in piece A
            RB = NPAD - RA
            ag_ins = [dr.tile([NPAD, NAG], BF16, tag=f"ag_in{l}", name=f"ag_in{l}")
                      if l else None for l in range(L)]
            ag_outsA = [dr.tile([NC * RA, NAG], BF16, tag=f"ag_outA{l}",
                                name=f"ag_outA{l}", addr_space="Shared")
                        if l else None for l in range(L)]
            ag_outsB = [dr.tile([NC * RB, NAG], BF16, tag=f"ag_outB{l}",
                                name=f"ag_outB{l}", addr_space="Shared")
                        if l else None for l in range(L)]
            gts = [dr.tile([NC * NPAD, P], BF16, tag=f"gt{l}", name=f"gt{l}")
                   if l else None for l in range(L)]

            # ---- issue ALL layer-0 gathers immediately (source table is the
            # external dict e_gt0; no AllGather needed)
            Gbs = {}

            def issue_gather(l, b):
                cb0 = int(off[b * GPB])
                CB = int(off[(b + 1) * GPB] - cb0)
                Gb = gp.tile([P, CB, NAG], BF16, tag=f"Gb{b}", bufs=1,
                             name=f"Gb{b}_{l}")
                src = e_gt0 if l == 0 else gts[l]
                idxs = idxw0 if l == 0 else idxw
                gi = nc.gpsimd.dma_gather(
                    out_ap=Gb[:], in_ap=src[:, 0:NAG],
                    idxs_ap=idxs[:, cb0 * 8:(cb0 + CB) * 8],
                    num_idxs=CB * P, num_idxs_reg=nig_regs[CB * P],
                    elem_size=NAG, elem_step=P, single_packet=False,
                    queue_num=GQ[b])
                add_dep_helper(gi.ins, rel.ins, False, "needs mlp lib")
                Gbs[b] = Gb

            for b in range(NB):
                issue_gather(0, b)

            # --- remaining loads (DMAs hide under the layer-0 gathers)
            iotac = load(w_iotac, [P, 1], "iotac")
            ident = load(w_ident, [P, P], "ident")
            T0 = load(w_t0, [P, NROW], "T0")
            Wconv = load(w_conv, [P, L * HID], "Wconv")
            Watt = load(w_att, [HID, L * 2], "Watt")
            idxw = load(e_idxw, [P, C * 8], "idxw", dtype=I16)
            AE = load(e_ae, [P, C, 6], "AE", dtype=BF16)
            aeloop = load(e_aeloop, [P, GRP, L], "aeloop")
            Wm1 = load(w_m1, [HID, L * HID], "Wm1")
            Wm2 = load(w_m2, [HID, L * HID], "Wm2")
            Wm3 = load(w_m3, [HID, L * DIM], "Wm3")
            B1 = load(w_b1, [HID, L], "B1")
            B2 = load(w_b2, [HID, L], "B2")
            B3 = load(w_b3, [DIM, L], "B3")
            Eps = load(w_eps, [P, L], "Eps")
            G1W1 = load(w_g1w1, [DIM, HID], "G1W1")
            G1B1 = load(w_g1b1, [HID, 1], "G1B1")
            G1W2 = load(w_g1w2, [HID, 1], "G1W2")
            G1B2 = load(w_g1b2, [1, 1], "G1B2")
            G2W1 = load(w_g2w1, [DIM, HID], "G2W1")
            G2B1 = load(w_g2b1, [HID, 1], "G2B1")
            G2W2 = load(w_g2w2, [HID, DIM], "G2W2")
            G2B2 = load(w_g2b2, [DIM, 1], "G2B2")
            PW1 = load(w_pw1, [DIM, HID], "PW1")
            PB1 = load(w_pb1, [HID, 1], "PB1")
            PW2 = load(w_pw2, [HID, HID], "PW2")
            PB2 = load(w_pb2, [HID, 1], "PB2")
            PW3 = load(w_pw3, [HID, 1], "PW3")
            PB3 = load(w_pb3, [1, 1], "PB3")
            Spool = load(e_spool, [P, GRP, 8], "Spool")
            Pcnt = load(e_pcnt, [8, 1], "Pcnt")

            identb = c1.tile([P, P], BF16, tag="identb")
            nc.vector.tensor_copy(out=identb[:], in_=ident[:])
            Wattb = c1.tile([HID, L * 2], BF16, tag="Wattb")
            nc.vector.tensor_copy(out=Wattb[:], in_=Watt[:])
            Spoolb = c1.tile([P, GRP, 8], BF16, tag="Spoolb")
            nc.vector.tensor_copy(out=Spoolb[:], in_=Spool[:])

            # float32r copies of matmul weights (DVE rounds on write, which the
            # BIR verifier requires of every fp32r-matmul producer)
            def rcopy(src, shape, tag):
                t = c1.tile(shape, F32R, tag=tag)
                nc.vector.tensor_copy(out=t[:], in_=src[:])
                return t

            T0r = rcopy(T0, [P, NROW], "T0r")
            Wconvr = rcopy(Wconv, [P, L * HID], "Wconvr")
            Wm1r = rcopy(Wm1, [HID, L * HID], "Wm1r")
            Wm2r = rcopy(Wm2, [HID, L * HID], "Wm2r")
            Wm3r = rcopy(Wm3, [HID, L * DIM], "Wm3r")
            Epsr = rcopy(Eps, [P, L], "Epsr")
            G1W1r = rcopy(G1W1, [DIM, HID], "G1W1r")
            G2W1r = rcopy(G2W1, [DIM, HID], "G2W1r")
            G1W2r = rcopy(G1W2, [HID, 1], "G1W2r")
            G2W2r = rcopy(G2W2, [HID, DIM], "G2W2r")

            ones1_128 = c1.tile([1, P], F32, tag="ones1_128")
            nc.vector.memset(ones1_128[:], 1.0)
            ones1r = c1.tile([1, P], F32R, tag="ones1r")
            nc.vector.tensor_copy(out=ones1r[:], in_=ones1_128[:])

            # x_idx broadcast to [128, NPAD] (partition-stride-0 DMA read)
            xidxb = c1.tile([P, NPAD], F32, tag="xbig")
            nc.sync.dma_start(out=xidxb[:], in_=AP(e_xidx, 0, [[0, P], [1, NPAD]]))

            outc = c1.tile([HID, NPAD], F32R, tag="outc")
            feat = c1.tile([P, NPAD], F32R, tag="feat")
            bn = c1.tile([1, NPAD], F32, tag="bn")      # running best jkn norm
            r0 = c1.tile([1, NPAD], F32R, tag="r0")     # is-better mask row
            h2T = c1.tile([DIM, NPAD], F32, tag="h2T")
            wrow = c1.tile([1, NPAD], F32, tag="wrow")
            pool = psP.tile([8, DIM + 1], F32, tag="pool")

            # ---- layer-0 node-major staging: nm0 = one-hot(x_idx) @ T0
            nm = stgp.tile([P, GRP, NROW], BF16, tag="nm", bufs=2, name="nm0")
            for s in range(5):
                sl = slice(s * 512, (s + 1) * 512)
                ohx = stgp.tile([P, 512], F32R, tag="stg", name="ohx")
                nc.vector.tensor_scalar(out=ohx[:], in0=xidxb[:, sl],
                                        scalar1=iotac[:], scalar2=None,
                                        op0=OP.is_equal)
                for t in range(4):
                    g = s * 4 + t
                    pnm = psB.tile([P, NROW], F32, tag="psB", name="pnm")
                    nc.tensor.matmul(out=pnm[:], lhsT=ohx[:, t * P:(t + 1) * P],
                                     rhs=T0r[:], start=True, stop=True)
                    nc.scalar.activation(nm[:, g, :], pnm[:], ACTF.Copy)

            def stage_slice(l, s, x_src, nm_t):
                """conv h | a_s | a_d for slice s of layer l -> node-major nm_t."""
                sl = slice(s * 512, (s + 1) * 512)
                ph = psB.tile([HID, 512], F32, tag="psB")
                nc.tensor.matmul(out=ph[:], lhsT=Wconvr[:, l * HID:(l + 1) * HID],
                                 rhs=x_src[:, sl], start=True, stop=True)
                stg = stgp.tile([NROW, 512], BF16, tag="stg")
                nc.scalar.activation(stg[0:HID, :], ph[:], ACTF.Copy)
                pa = psB.tile([2, 512], F32, tag="psB")
                nc.tensor.matmul(out=pa[:], lhsT=Wattb[:, l * 2:(l + 1) * 2],
                                 rhs=stg[0:HID, :], start=True, stop=True)
                nc.scalar.activation(stg[HID:HID + 2, :], pa[:], ACTF.Copy)
                for t in range(4):
                    g = s * 4 + t
                    ptr = psA.tile([P, NROW], BF16, tag="psA")
                    nc.tensor.transpose(out=ptr[:], in_=stg[:, t * 128:(t + 1) * 128],
                                        identity=identb[:NROW, :NROW])
                    if t % 2 == 0:
                        nc.vector.tensor_copy(out=nm_t[:, g, :], in_=ptr[:])
                    else:
                        nc.scalar.activation(nm_t[:, g, :], ptr[:], ACTF.Copy)
                # ship this slice's 4 groups to the comm buffer
                ago = ag_ins[l][:].rearrange("(g p) c -> p g c", p=P)
                nc.sync.dma_start(out=ago[:, s * 4:(s + 1) * 4, :],
                                  in_=nm_t[:, s * 4:(s + 1) * 4, 0:65])

            def issue_ag_a(l):
                nc.gpsimd.collective_compute(
                    "AllGather", OP.bypass, replica_groups=[list(range(NC))],
                    ins=[ag_ins[l][0:RA, :]], outs=[ag_outsA[l][:]])
                gv = gts[l][:].rearrange("(r n) c -> r n c", n=NPAD)
                av = ag_outsA[l][:].rearrange("(r n) c -> r n c", n=RA)
                # split the 256B-row expansion copy across both HWDGE engines
                nc.sync.dma_start(out=gv[0:4, 0:RA, 0:65], in_=av[0:4])
                nc.scalar.dma_start(out=gv[4:8, 0:RA, 0:65], in_=av[4:8])

            def issue_ag_b(l):
                nc.gpsimd.collective_compute(
                    "AllGather", OP.bypass, replica_groups=[list(range(NC))],
                    ins=[ag_ins[l][RA:NPAD, :]], outs=[ag_outsB[l][:]])
                gv = gts[l][:].rearrange("(r n) c -> r n c", n=NPAD)
                av = ag_outsB[l][:].rearrange("(r n) c -> r n c", n=RB)
                nc.sync.dma_start(out=gv[0:4, RA:NPAD, 0:65], in_=av[0:4])
                nc.scalar.dma_start(out=gv[4:8, RA:NPAD, 0:65], in_=av[4:8])

            def jkn_slice(l, s, xl):
                sl = slice(s * 512, (s + 1) * 512)
                sq = sm.tile([P, 512], F32R, tag="sq", name="sq", bufs=2)
                nc.scalar.activation(sq[:], _f(xl[:, sl]), ACTF.Square)
                pml = psB.tile([1, 512], F32, tag="psB", name="pml")
                nc.tensor.matmul(out=pml[:], lhsT=Epsr[:, l:l + 1], rhs=sq[:],
                                 start=True, stop=True)
                if l == 0:
                    nc.vector.tensor_copy(out=bn[0:1, sl], in_=pml[:])
                    nc.vector.tensor_copy(out=feat[:, sl], in_=_f(xl[:, sl]))
                else:
                    nc.vector.tensor_tensor(out=r0[0:1, sl], in0=pml[:],
                                            in1=bn[0:1, sl], op=OP.is_gt)
                    nc.vector.tensor_tensor(out=bn[0:1, sl], in0=bn[0:1, sl],
                                            in1=pml[:], op=OP.max)
                    pm = psB.tile([P, 512], F32, tag="psB", name="pm")
                    nc.tensor.matmul(out=pm[:], lhsT=ones1r[:], rhs=r0[0:1, sl],
                                     start=True, stop=True)
                    ft = sm.tile([P, 512], F32, tag="ft", name="ft", bufs=2)
                    nc.vector.tensor_tensor(out=ft[:], in0=_f(xl[:, sl]),
                                            in1=_f(feat[:, sl]), op=OP.subtract)
                    nc.vector.tensor_tensor(out=ft[:], in0=ft[:], in1=pm[:],
                                            op=OP.mult)
                    nc.vector.tensor_tensor(out=feat[:, sl], in0=_f(feat[:, sl]),
                                            in1=ft[:], op=OP.add)

            def pool_slice(s):
                """Graph-pooling feature matmuls for node slice s (from feat)."""
                sl = slice(s * 512, (s + 1) * 512)
                pa1 = psB.tile([HID, 512], F32, tag="psB")
                nc.tensor.matmul(out=pa1[:], lhsT=G1W1r[:], rhs=feat[:, sl],
                                 start=True, stop=True)
                r1 = yp.tile([HID, 512], F32R, tag="y", name="r1", bufs=3)
                nc.scalar.activation(r1[:], pa1[:], ACTF.Relu, bias=G1B1[:])
                ph1 = psB.tile([1, 512], F32, tag="psB", name="ph1")
                nc.tensor.matmul(out=ph1[:], lhsT=G1W2r[:], rhs=r1[:],
                                 start=True, stop=True)
                nc.scalar.activation(wrow[0:1, sl], ph1[:], ACTF.Exp, bias=G1B2[:])
                pa2 = psB.tile([HID, 512], F32, tag="psB")
                nc.tensor.matmul(out=pa2[:], lhsT=G2W1r[:], rhs=feat[:, sl],
                                 start=True, stop=True)
                r2 = yp.tile([HID, 512], F32R, tag="y", name="r2", bufs=3)
                nc.scalar.activation(r2[:], pa2[:], ACTF.Relu, bias=G2B1[:])
                ph2 = psB.tile([DIM, 512], F32, tag="psB")
                nc.tensor.matmul(out=ph2[:], lhsT=G2W2r[:], rhs=r2[:],
                                 start=True, stop=True)
                nc.scalar.activation(h2T[:, sl], ph2[:], ACTF.Identity, bias=G2B2[:])

            x_cur = None
            pending_jkn = None
            # =================== layers ===================
            for l in range(L):
                if pending_jkn is not None:
                    pending_jkn()
                    pending_jkn = None
                # ---- self-loop weights, node-major [128, GRP]
                wloop = sm.tile([P, GRP], F32, tag="wloop")
                zt = sm.tile([P, GRP], F32, tag="zt")
                nc.vector.tensor_tensor(out=zt[:], in0=nm[:, :, 64], in1=nm[:, :, 65],
                                        op=OP.add)
                nc.vector.tensor_tensor(out=zt[:], in0=zt[:], in1=aeloop[:, :, l],
                                        op=OP.add)
                t2 = sm.tile([P, GRP], F32, tag="zt2")
                nc.scalar.activation(t2[:], zt[:], ACTF.Prelu, alpha=LRELU)
                nc.scalar.activation(wloop[:], t2[:], ACTF.Exp)
                smsg_all = sm.tile([P, GRP, 64], BF16, tag="smsg_all", bufs=1)
                nc.vector.tensor_tensor(out=smsg_all[:], in0=nm[:, :, 0:64],
                                        in1=_bc(wloop[:], 2, 64), op=OP.mult)
                # ---- per-group edge processing for one gather batch
                def edge_batch(b, l=l, nm=nm, wloop=wloop, smsg_all=smsg_all):
                    cb0 = int(off[b * GPB])
                    Gb = Gbs[b]
                    for gg in range(GPB):
                        g = b * GPB + gg
                        Mg = Ms[g]
                        gs = int(off[g])
                        cb = gs - cb0
                        # logits -> normalized weights (ACT-heavy chain)
                        z = zp.tile([P, MMAX], F32, tag="z")
                        nc.vector.scalar_tensor_tensor(
                            out=z[:, 0:Mg], in0=Gb[:, cb:cb + Mg, 64],
                            scalar=nm[:, g, 65:66], in1=AE[:, gs:gs + Mg, l],
                            op0=OP.add, op1=OP.add)
                        zl = zp.tile([P, MMAX], F32, tag="t0")
                        nc.scalar.activation(zl[:, 0:Mg], z[:, 0:Mg], ACTF.Prelu,
                                             alpha=LRELU)
                        wb = zp.tile([P, MMAX], BF16, tag="wb")
                        dn0 = sm.tile([P, 1], F32, tag="dn", bufs=2)
                        nc.scalar.activation(wb[:, 0:Mg], zl[:, 0:Mg], ACTF.Exp,
                                             accum_out=dn0[:])
                        dn = sm.tile([P, 1], F32, tag="dnf", bufs=2)
                        nc.vector.tensor_tensor(out=dn[:], in0=dn0[:],
                                                in1=wloop[:, g:g + 1], op=OP.add)
                        rec = sm.tile([P, 1], F32, tag="rec")
                        nc.vector.reciprocal(out=rec[:], in_=dn[:])
                        wb2 = zp.tile([P, MMAX, 2], BF16, tag="wb2")
                        nc.scalar.activation(wb2[:, 0:Mg, :],
                                             _bc(wb[:, 0:Mg], 1, 2), ACTF.Copy)
                        gsl = Gb[:, cb:cb + Mg, 0:64]
                        gv = AP(gsl.tensor, gsl.offset,
                                [list(gsl.ap[0]), [65, Mg], [2, 32], [1, 2]])
                        w2 = wb2[:, 0:Mg, :]
                        wv = AP(w2.tensor, w2.offset,
                                [list(w2.ap[0]), [2, Mg], [0, 32], [1, 2]])
                        nc.vector.tensor_tensor(out=gv, in0=gv, in1=wv, op=OP.mult)
                        # scatter-accumulate (node-major)
                        pg = psA.tile([P, 64], F32, tag="psA")
                        for k in range(Mg):
                            nc.tensor.matmul(out=pg[:], lhsT=identb[:],
                                             rhs=Gb[:, cb + k, 0:64],
                                             start=(k == 0), stop=False)
                        nc.tensor.matmul(out=pg[:], lhsT=identb[:],
                                         rhs=smsg_all[:, g, :],
                                         start=False, stop=True)
                        # normalize on ACT (PSUM read) + transpose to feature-major
                        onm = sm.tile([P, 64], F32, tag="onm")
                        nc.scalar.activation(onm[:], pg[:, 0:64], ACTF.Copy,
                                             scale=rec[:])
                        ptr2 = psA.tile([64, P], F32, tag="psA")
                        nc.tensor.transpose(out=ptr2[:], in_=onm[:], identity=ident[:])
                        nc.vector.tensor_copy(out=outc[:, g * P:(g + 1) * P],
                                              in_=ptr2[:])

                # ---- MLP (feature-major), fused with next-layer staging per slice
                x_new = c1.tile([P, NPAD], F32R, tag=f"xl{l % 2}", name=f"x{l}")
                if l < L - 1:
                    nm_next = stgp.tile([P, GRP, NROW], BF16, tag="nm", bufs=2,
                                        name=f"nm{l + 1}")
                else:
                    nm_next = None

                def mlp_slice(s, l=l, x_new=x_new, nm_next=nm_next):
                    sl = slice(s * 512, (s + 1) * 512)
                    p1 = psB.tile([HID, 512], F32, tag="psB")
                    nc.tensor.matmul(out=p1[:], lhsT=Wm1r[:, l * HID:(l + 1) * HID],
                                     rhs=outc[:, sl], start=True, stop=True)
                    y1 = yp.tile([HID, 512], F32R, tag="y", name="y1", bufs=3)
                    nc.scalar.activation(y1[:], p1[:], ACTF.Relu, bias=B1[:, l:l + 1])
                    p2 = psB.tile([HID, 512], F32, tag="psB")
                    nc.tensor.matmul(out=p2[:], lhsT=Wm2r[:, l * HID:(l + 1) * HID],
                                     rhs=y1[:], start=True, stop=True)
                    y2 = yp.tile([HID, 512], F32R, tag="y", name="y2", bufs=3)
                    nc.scalar.activation(y2[:], p2[:], ACTF.Relu, bias=B2[:, l:l + 1])
                    p3 = psB.tile([P, 512], F32, tag="psB")
                    nc.tensor.matmul(out=p3[:], lhsT=Wm3r[:, l * DIM:(l + 1) * DIM],
                                     rhs=y2[:], start=True, stop=True)
                    nc.scalar.activation(x_new[:, sl], p3[:], ACTF.Identity,
                                         bias=B3[:, l:l + 1])
                    if nm_next is not None:
                        stage_slice(l + 1, s, x_new, nm_next)
                    else:
                        # last layer: fold JKN + pooling feature matmuls in here
                        jkn_slice(l, s, x_new)
                        pool_slice(s)

                # interleave: MLP slice s fires right after its aligned batch
                edge_batch(0)
                edge_batch(1)
                mlp_slice(0)
                edge_batch(2)
                mlp_slice(1)
                edge_batch(3)
                mlp_slice(2)
                if l < L - 1:
                    issue_ag_a(l + 1)
                edge_batch(4)
                mlp_slice(3)
                mlp_slice(4)
                if l < L - 1:
                    issue_ag_b(l + 1)
                    for b in range(NB):
                        issue_gather(l + 1, b)
                x_cur = x_new
                nm = nm_next

                if l < L - 1:
                    def _jkn_update(xl=x_new, l=l):
                        for s in range(5):
                            jkn_slice(l, s, xl)
                    pending_jkn = _jkn_update

            # =================== pooling (node-major, permutation-proof) ===================
            for g in range(GRP):
                cs = slice(g * P, (g + 1) * P)
                # node-major exp weights and h2 for this 128-node chunk
                pwn = psA.tile([P, 1], F32, tag="psA", name="pwn")
                nc.tensor.transpose(out=pwn[:], in_=wrow[0:1, cs], identity=ident[0:1, 0:1])
                wn = sm.tile([P, 1], F32, tag="wn", bufs=2)
                nc.vector.tensor_copy(out=wn[:], in_=pwn[:])
                ph2n = psA.tile([P, DIM], F32, tag="psA", name="ph2n")
                nc.tensor.transpose(out=ph2n[:], in_=h2T[:, cs], identity=ident[:])
                whn = sm.tile([P, DIM + 1], BF16, tag="whn", bufs=2)
                nc.vector.tensor_scalar(out=whn[:, 0:DIM], in0=ph2n[:], scalar1=wn[:],
                                        scalar2=None, op0=OP.mult)
                nc.vector.tensor_copy(out=whn[:, DIM:DIM + 1], in_=wn[:])
                nc.tensor.matmul(out=pool[:], lhsT=Spoolb[:, g, :], rhs=whn[:],
                                 start=(g == 0), stop=(g == GRP - 1))

            pool_sb = sm.tile([8, DIM + 1], F32, tag="pool_sb")
            nc.vector.tensor_copy(out=pool_sb[:], in_=pool[:])
            den = sm.tile([8, 1], F32, tag="den")
            nc.vector.tensor_tensor(out=den[:], in0=pool_sb[:, DIM:DIM + 1],
                                    in1=Pcnt[:], op=OP.mult)
            rcp = sm.tile([8, 1], F32, tag="rcp")
            nc.vector.reciprocal(out=rcp[:], in_=den[:])
            nc.vector.tensor_scalar(out=pool_sb[:, 0:DIM], in0=pool_sb[:, 0:DIM],
                                    scalar1=rcp[:], scalar2=None, op0=OP.mult)
            phg = psA.tile([DIM, 8], F32, tag="psA", name="phg")
            nc.tensor.transpose(out=phg[:], in_=pool_sb[:, 0:DIM], identity=ident[:8, :8])
            hg = c1.tile([DIM, 8], F32, tag="hg")
            nc.vector.tensor_copy(out=hg[:], in_=phg[:])

            pp1 = psB.tile([HID, 8], F32, tag="psB")
            nc.tensor.matmul(out=pp1[:], lhsT=PW1[:], rhs=hg[:], start=True, stop=True)
            rp1 = sm.tile([HID, 8], F32, tag="rp1")
            nc.scalar.activation(rp1[:], pp1[:], ACTF.Relu, bias=PB1[:])
            pp2 = psB.tile([HID, 8], F32, tag="psB")
            nc.tensor.matmul(out=pp2[:], lhsT=PW2[:], rhs=rp1[:], start=True, stop=True)
            rp2 = sm.tile([HID, 8], F32, tag="rp2")
            nc.scalar.activation(rp2[:], pp2[:], ACTF.Relu, bias=PB2[:])
            pp3 = psB.tile([1, 8], F32, tag="psB")
            nc.tensor.matmul(out=pp3[:], lhsT=PW3[:], rhs=rp2[:], start=True, stop=True)
            ores = sm.tile([1, 8], F32, tag="ores")
            nc.vector.tensor_scalar(out=ores[:], in0=pp3[:], scalar1=PB3[:],
                                    scalar2=None, op0=OP.add)
            nc.sync.dma_start(out=eout[:], in_=ores[:])
            ps_stackP.__exit__(None, None, None)
            ps_stackB.__exit__(None, None, None)
            ps_stack.__exit__(None, None, None)

    lower_extended_insts(nc)
    return nc


def _prep_host(inputs):
    src = np.asarray(inputs['edge_index'][0]).astype(np.int64)
    dst = np.asarray(inputs['edge_index'][1]).astype(np.int64)
    attr = np.asarray(inputs['edge_attr_idx']).astype(np.int64)
    x_idx = np.asarray(inputs['x_idx']).astype(np.int64)
    batch = np.asarray(inputs['batch']).astype(np.int64)
    emb = np.asarray(inputs['emb']).astype(np.float32)

    conv_W = np.asarray(inputs['conv_W'], np.float32)
    conv_We = np.asarray(inputs['conv_We'], np.float32)
    att_s = np.asarray(inputs['conv_att_src'], np.float32)
    att_d = np.asarray(inputs['conv_att_dst'], np.float32)
    att_e = np.asarray(inputs['conv_att_edge'], np.float32)
    V = np.stack([conv_We[l] @ att_e[l] for l in range(L)], 1)    # [128, 6]
    t_all = (emb @ V).astype(np.float32)                          # [128, 6]

    owner = dst // NPC

    # per-core edge lists + in-degree-sorted permutations
    per_core = []
    perms = []     # perm[c][slot] = original local node (len NPC for slots < NPC)
    invs = np.zeros((NC, NPC), np.int64)
    degs = []
    for c in range(NC):
        m = np.where(owner == c)[0]
        dl = (dst[m] - c * NPC).astype(np.int64)
        counts = np.bincount(dl, minlength=NPC)
        order = np.argsort(-counts, kind='stable')  # degree desc
        inv = np.zeros(NPC, np.int64)
        inv[order] = np.arange(NPC)
        perms.append(order)
        invs[c] = inv
        degs.append(counts)
        per_core.append((m, dl, counts))

    # common per-group chunk counts (max across cores), in degree-sorted order
    Ms_sorted = np.zeros(GRP, np.int64)
    for c in range(NC):
        cs = degs[c][perms[c]]
        cs_pad = np.zeros(NPAD, np.int64)
        cs_pad[:NPC] = cs
        Ms_sorted = np.maximum(Ms_sorted, cs_pad.reshape(GRP, P).max(1))

    # pack quantile groups into NB batches with DESCENDING batch sums so the
    # last batch (whose consumers gate next-layer staging) is smallest
    GPB_ = GRP // NB
    Ctot = int(Ms_sorted.sum())
    frac = np.array([0.24, 0.22, 0.20, 0.18, 0.16])
    targets = frac * Ctot
    order_q = sorted(range(GRP), key=lambda i: -int(Ms_sorted[i]))
    batches = [[] for _ in range(NB)]
    bsums = [0.0] * NB
    for qg in order_q:
        cands = [j for j in range(NB) if len(batches[j]) < GPB_]
        i = min(cands, key=lambda j: (bsums[j] + Ms_sorted[qg]) / targets[j])
        batches[i].append(qg)
        bsums[i] += int(Ms_sorted[qg])
    border = sorted(range(NB), key=lambda j: -bsums[j])
    order_groups = [qg for j in border for qg in batches[j]]
    Ms = [int(Ms_sorted[qg]) for qg in order_groups]
    off = np.zeros(GRP + 1, np.int64)
    np.cumsum(Ms, out=off[1:])
    C = int(off[GRP])

    # slot permutation: slot j*128+p  <- degree-rank order_groups[j]*128+p
    slot_of_rank = np.zeros(NPAD, np.int64)   # rank -> slot
    for j, qg in enumerate(order_groups):
        slot_of_rank[qg * P:(qg + 1) * P] = np.arange(j * P, (j + 1) * P)

    # node_at_slot[c][slot] = original local node id (or -1 for pads)
    node_at_slot = np.full((NC, NPAD), -1, np.int64)
    for c in range(NC):
        ranks = np.arange(NPC)
        node_at_slot[c, slot_of_rank[ranks]] = perms[c]

    # global padded slot id of each source node (for gather indices)
    src_slot = np.zeros(N, np.int64)
    for c in range(NC):
        loc = np.arange(NPC)
        src_slot[c * NPC + loc] = c * NPAD + slot_of_rank[invs[c][loc]]

    def wrap16(fl):
        w = np.zeros((P, fl.shape[0] // 16), np.int16)
        for r in range(16):
            w[r::16, :] = fl[r::16].reshape(1, -1)
        return w

    import ml_dtypes
    cores = []
    for c in range(NC):
        m, dl, counts = per_core[c]
        # edges sorted by dst for segment extraction
        order_e = np.argsort(dl, kind='stable')
        eidx = m[order_e]
        dls = dl[order_e]
        starts = np.zeros(NPC + 1, np.int64)
        np.cumsum(np.bincount(dls, minlength=NPC), out=starts[1:])

        idxflat = np.zeros(C * P, np.int64)
        idxflat0 = np.zeros(C * P, np.int64)
        ae = np.full((P, C, 6), -1e9, np.float32)
        for slot_g in range(GRP):
            base = int(off[slot_g])
            Mg = Ms[slot_g]
            for p in range(P):
                slot = slot_g * P + p
                n = node_at_slot[c, slot]
                if n < 0:
                    continue
                s0, cnt = starts[n], counts[n]
                assert cnt <= Mg, (c, slot_g, p, cnt, Mg)
                es = eidx[s0:s0 + cnt]
                for k in range(cnt):
                    ch = base + k
                    idxflat[ch * P + p] = src_slot[src[es[k]]]
                    idxflat0[ch * P + p] = x_idx[src[es[k]]]
                    ae[p, ch, 0:6] = t_all[attr[es[k]]]
        idxw = wrap16(idxflat.astype(np.int16))
        idxw0 = wrap16(idxflat0.astype(np.int16))
        # per-node loop attr, permuted node-major
        ae_sum = np.zeros((NPC, L), np.float32)
        np.add.at(ae_sum, dls, t_all[attr[eidx]])
        ael = ae_sum / np.maximum(counts.astype(np.float32), 1.0)[:, None]
        ael_slot = np.zeros((NPAD, L), np.float32)
        valid = node_at_slot[c] >= 0
        ael_slot[valid] = ael[node_at_slot[c][valid]]
        aeloop = ael_slot.reshape(GRP, P, L).transpose(1, 0, 2).copy()
        # pooling one-hot [P, GRP, 8] and x_idx per slot
        spool = np.zeros((NPAD, 8), np.float32)
        gids = batch[c * NPC + node_at_slot[c][valid]] - c * 8
        spool[np.where(valid)[0], gids] = 1.0
        spool = spool.reshape(GRP, P, 8).transpose(1, 0, 2).copy()
        xi = np.full(NPAD, -1.0, np.float32)
        xi[valid] = x_idx[c * NPC + node_at_slot[c][valid]].astype(np.float32)
        cnts = np.bincount(batch[c * NPC:(c + 1) * NPC] - c * 8,
                           minlength=8).astype(np.float32)
        cores.append(dict(e_idxw=idxw, e_idxw0=idxw0,
                          e_ae=ae.astype(ml_dtypes.bfloat16),
                          e_aeloop=aeloop, e_spool=spool,
                          e_xidx=xi, e_pcnt=cnts.reshape(8, 1)))

    # ---- shared weights
    conv_b = np.asarray(inputs['conv_b'], np.float32)
    m1 = np.asarray(inputs['mlp_W1'], np.float32)
    m2 = np.asarray(inputs['mlp_W2'], np.float32)
    m3 = np.asarray(inputs['mlp_W3'], np.float32)
    b1 = np.asarray(inputs['mlp_b1'], np.float32)
    b2 = np.asarray(inputs['mlp_b2'], np.float32)
    b3 = np.asarray(inputs['mlp_b3'], np.float32)
    b1_eff = np.stack([conv_b[l] @ m1[l] + b1[l] for l in range(L)], 1)

    # layer-0 dictionary: h0 = emb@W0, a_s0/a_d0 per embedding id
    h0 = emb @ conv_W[0]                      # [128, 64]
    a_s0 = h0 @ att_s[0]
    a_d0 = h0 @ att_d[0]
    gt0 = np.zeros((P, P), np.float32)
    gt0[:, 0:HID] = h0
    gt0[:, HID] = a_s0
    t0 = np.zeros((P, NROW), np.float32)
    t0[:, 0:HID] = h0
    t0[:, HID] = a_s0
    t0[:, HID + 1] = a_d0

    shared = dict(
        w_iotac=np.arange(P, dtype=np.float32).reshape(P, 1),
        w_ident=np.eye(P, dtype=np.float32),
        e_gt0=gt0.astype(ml_dtypes.bfloat16),
        w_t0=t0,
        w_conv=np.concatenate([conv_W[l] for l in range(L)], 1),
        w_att=np.concatenate([np.stack([att_s[l], att_d[l]], 1) for l in range(L)], 1),
        w_m1=np.concatenate([m1[l] for l in range(L)], 1),
        w_m2=np.concatenate([m2[l] for l in range(L)], 1),
        w_m3=np.concatenate([m3[l] for l in range(L)], 1),
        w_b1=b1_eff,
        w_b2=b2.T.copy(),
        w_b3=b3.T.copy(),
        w_eps=np.broadcast_to((1.0 - np.arange(L, dtype=np.float32) * 1e-7)[None, :],
                              (P, L)).copy(),
        w_g1w1=np.asarray(inputs['g1_W1'], np.float32),
        w_g1b1=np.asarray(inputs['g1_b1'], np.float32).reshape(HID, 1),
        w_g1w2=np.asarray(inputs['g1_W2'], np.float32),
        w_g1b2=np.asarray(inputs['g1_b2'], np.float32).reshape(1, 1),
        w_g2w1=np.asarray(inputs['g2_W1'], np.float32),
        w_g2b1=np.asarray(inputs['g2_b1'], np.float32).reshape(HID, 1),
        w_g2w2=np.asarray(inputs['g2_W2'], np.float32),
        w_g2b2=np.asarray(inputs['g2_b2'], np.float32).reshape(DIM, 1),
        w_pw1=np.asarray(inputs['p_W1'], np.float32),
        w_pb1=np.asarray(inputs['p_b1'], np.float32).reshape(HID, 1),
        w_pw2=np.asarray(inputs['p_W2'], np.float32),
        w_pb2=np.asarray(inputs['p_b2'], np.float32).reshape(HID, 1),
        w_pw3=np.asarray(inputs['p_W3'], np.float32),
        w_pb3=np.asarray(inputs['p_b3'], np.float32).reshape(1, 1),
    )

    in_maps = []
    for c in range(NC):
        mm = dict(shared)
        mm.update(cores[c])
        in_maps.append(mm)
    return tuple(Ms), in_maps


def kernel(**inputs):
    Ms, in_maps = _prep_host(inputs)
    if Ms not in _cache:
        _cache[Ms] = _build(Ms)
    nc = _cache[Ms]
    res = run_bass_kernel_spmd(nc, in_maps, core_ids=list(range(NC)))
    out = np.concatenate([np.asarray(res.results[c]['out']).reshape(8)
                          for c in range(NC)])
    return out.astype(np.float32)


if __name__ == "__main__":
    import jax
    sys.path.insert(0, '/root/problem')
    import reference as R
    with jax.default_device(jax.devices('cpu')[0]):
        inp = R.setup_inputs()
        exp = np.asarray(R.reference(**inp))
    inp = {k: np.asarray(v) for k, v in inp.items()}
    act = kernel(**inp)
    rel = np.linalg.norm(act - exp) / np.linalg.norm(exp)
    print("Relative error:", rel)


# revision 25
# speedup vs baseline: 1.0729x; 1.0729x over previous
"""Trainium2 Bass kernel for nn_GAT_34059090657327 (6-layer GAT + JKN + attention pooling).

V4 on top of V3:
  - Gathers rotate across SWDGE queues 1-3 (num_swdge_queues=4): descriptor
    generation for queues >0 runs asynchronously on idle Q7 core pairs, tripling
    effective gather throughput (~25us vs ~76us per 8.5k-idx gather).
  - Layer 0 gathers from a 128-row dictionary table (emb@W0|a_s precomputed on
    host) using x_idx[src] indices: no AllGather(0), no x_init one-hot matmuls.
    nm0 staging is 20 one-hot matmuls against a [128,66] table.
  - Per-layer AllGather split in two (slices 0-2 after stage 2, slices 3-4 at
    layer end) so most of the exchange hides under the gather stream.
  - Per-group softmax chain moved to the ACT engine (Lrelu, Exp with accum_out
    denominator, normalize via PSUM-side Copy*scale) - the DVE was being
    port-starved by SWDGE descriptor traffic.
  - Gather batches sized descending so the last batch's consumers (which gate
    staging of the next layer) finish sooner.
  - Layer-5 JKN + pooling feature matmuls run inside the layer-5 MLP slices.
Distribution: dst-sharded nodes+edges, per-layer compact bf16 AllGather of the
node table (h|a_s), local expand to 256B rows, identity-matmul scatter per group.
"""
import numpy as np
import sys

sys.path.insert(0, '/opt/trn_rl_repo')

import concourse.bass as bass
import concourse.mybir as mybir
import concourse.tile as tile
from concourse import library_config
from concourse.bass import AP
from concourse.bass_utils import run_bass_kernel_spmd
from concourse.library_overlay import lower_extended_insts
from concourse.tile_rust import add_dep_helper

F32 = mybir.dt.float32
F32R = mybir.dt.float32r
BF16 = mybir.dt.bfloat16
I16 = mybir.dt.int16
OP = mybir.AluOpType
ACTF = mybir.ActivationFunctionType

N, E, NG, DIM, HID, L = 20000, 320000, 64, 128, 64, 6
NC = 8
NPC = N // NC            # 2500
P = 128
GRP = 20                 # node groups of 128 per core
NPAD = GRP * P           # 2560
LRELU = 0.2
NROW = 66                # compact node-table row: h(64) | a_s | a_d
NB = 5                   # gather batches per layer (GRP/NB groups each)
GQ = (1, 2, 3, 1, 2)     # SWDGE queue per gather batch (queues 1-3 run async)
AGS = 3                  # slices covered by the early AllGather piece

_cache = {}

# ---------------------------------------------------------------------------
# This walrus build encodes only ONE semaphore wait/update per TPB_CTRL
# instruction ("Too many sync wait commands" on the Tile tail drain). Split
# extra waits onto preceding NoOps at BIR-serialization time.
import json as _json


def _split_multiwaits(js: bytes) -> bytes:
    j = _json.loads(js)
    n = 0
    for fn in j["functions"]:
        for bb in fn["blocks"]:
            out = []
            for inst in bb["instructions"]:
                si = inst.get("sync_info") or {}
                waits = si.get("on_wait") or []
                if len(waits) > 1:
                    for w in waits[:-1]:
                        n += 1
                        out.append({
                            "name": inst["name"] + f"_w{n}", "opcode": "NoOp",
                            "engine": inst["engine"], "ins": [], "outs": [],
                            "sync_info": {"on_wait": [w], "on_update": []},
                        })
                    si["on_wait"] = [waits[-1]]
                out.append(inst)
                ups = si.get("on_update") or []
                if len(ups) > 1 and inst["opcode"] in ("NoOp", "Drain", "EventSemaphore"):
                    si["on_update"] = [ups[0]]
                    for u in ups[1:]:
                        n += 1
                        out.append({
                            "name": inst["name"] + f"_u{n}", "opcode": "NoOp",
                            "engine": inst["engine"], "ins": [], "outs": [],
                            "sync_info": {"on_wait": [], "on_update": [u]},
                        })
            bb["instructions"] = out
    return _json.dumps(j).encode()


if not getattr(bass.Bass, "_mw_patched", False):
    _orig_to_json_bytes = bass.Bass.to_json_bytes

    def _to_json_bytes_patched(self, *a, **k):
        return _split_multiwaits(_orig_to_json_bytes(self, *a, **k))

    bass.Bass.to_json_bytes = _to_json_bytes_patched
    bass.Bass._mw_patched = True



# ---------------------------------------------------------------------------
# Relax dma_gather's 256B elem-SIZE restriction for non-transpose gathers (the
# bass assert notes it is a "transpose restriction"; the element STRIDE stays
# 256B-aligned, which is the actual ISA field granularity). Installed as a
# textual patch of the original method so upstream changes surface loudly.
import inspect as _inspect

if not getattr(bass.BassGpSimd, "_gather_relaxed", False):
    _gsrc = _inspect.getsource(bass.BassGpSimd.dma_gather)
    _old_assert = (
        "assert (\n"
        "            elem_size_bytes > 0 and elem_size_bytes % 256 == 0\n"
        "        )  # transpose restriction"
    )
    assert _old_assert in _gsrc, "dma_gather source changed; re-derive patch"
    _gsrc = _gsrc.replace(
        _old_assert,
        "assert elem_size_bytes > 0 and (\n"
        "            elem_size_bytes % 256 == 0 or (not transpose and elem_size_bytes % 2 == 0)\n"
        "        )",
    )
    _gsrc = "def _dma_gather_relaxed" + _gsrc[_gsrc.index("("):]
    _ns = vars(bass).copy()
    exec(compile(_gsrc, "<dma_gather_relaxed>", "exec"), _ns)
    bass.BassGpSimd.dma_gather = _ns["_dma_gather_relaxed"]
    bass.BassGpSimd._gather_relaxed = True


def _bc(ap, pos, count):
    """Insert a stride-0 (broadcast) dim of `count` at free-dim position `pos`."""
    lst = [list(x) for x in ap.ap]
    lst.insert(1 + pos, [0, count])
    return AP(ap.tensor, ap.offset, lst)


def _f(ap):
    """View a float32r AP as plain fp32 for DVE/ACT consumers."""
    return ap.bitcast(F32)


def _build(Ms):
    Ms = list(Ms)
    assert len(Ms) == GRP
    off = np.zeros(GRP + 1, np.int64)
    np.cumsum(Ms, out=off[1:])
    C = int(off[GRP])         # total chunks per core
    GPB = GRP // NB           # groups per gather batch
    MMAX = max(Ms)
    CBMAX = max(int(off[(b + 1) * GPB] - off[b * GPB]) for b in range(NB))

    nc = bass.Bass(num_devices=NC, num_swdge_queues=4)

    # ---------------- inputs ----------------
    e_idxw = nc.dram_tensor("e_idxw", [P, C * 8], I16, kind="ExternalInput")
    e_idxw0 = nc.dram_tensor("e_idxw0", [P, C * 8], I16, kind="ExternalInput")
    e_gt0 = nc.dram_tensor("e_gt0", [P, P], BF16, kind="ExternalInput")
    e_ae = nc.dram_tensor("e_ae", [P, C, 6], BF16, kind="ExternalInput")
    e_aeloop = nc.dram_tensor("e_aeloop", [P, GRP, L], F32, kind="ExternalInput")
    e_spool = nc.dram_tensor("e_spool", [P, GRP, 8], F32, kind="ExternalInput")
    e_pcnt = nc.dram_tensor("e_pcnt", [8, 1], F32, kind="ExternalInput")
    e_xidx = nc.dram_tensor("e_xidx", [NPAD], F32, kind="ExternalInput")
    w_iotac = nc.dram_tensor("w_iotac", [P, 1], F32, kind="ExternalInput")
    w_ident = nc.dram_tensor("w_ident", [P, P], F32, kind="ExternalInput")
    w_t0 = nc.dram_tensor("w_t0", [P, NROW], F32, kind="ExternalInput")
    w_conv = nc.dram_tensor("w_conv", [P, L * HID], F32, kind="ExternalInput")
    w_att = nc.dram_tensor("w_att", [HID, L * 2], F32, kind="ExternalInput")
    w_m1 = nc.dram_tensor("w_m1", [HID, L * HID], F32, kind="ExternalInput")
    w_m2 = nc.dram_tensor("w_m2", [HID, L * HID], F32, kind="ExternalInput")
    w_m3 = nc.dram_tensor("w_m3", [HID, L * DIM], F32, kind="ExternalInput")
    w_b1 = nc.dram_tensor("w_b1", [HID, L], F32, kind="ExternalInput")
    w_b2 = nc.dram_tensor("w_b2", [HID, L], F32, kind="ExternalInput")
    w_b3 = nc.dram_tensor("w_b3", [DIM, L], F32, kind="ExternalInput")
    w_eps = nc.dram_tensor("w_eps", [P, L], F32, kind="ExternalInput")
    w_g1w1 = nc.dram_tensor("w_g1w1", [DIM, HID], F32, kind="ExternalInput")
    w_g1b1 = nc.dram_tensor("w_g1b1", [HID, 1], F32, kind="ExternalInput")
    w_g1w2 = nc.dram_tensor("w_g1w2", [HID, 1], F32, kind="ExternalInput")
    w_g1b2 = nc.dram_tensor("w_g1b2", [1, 1], F32, kind="ExternalInput")
    w_g2w1 = nc.dram_tensor("w_g2w1", [DIM, HID], F32, kind="ExternalInput")
    w_g2b1 = nc.dram_tensor("w_g2b1", [HID, 1], F32, kind="ExternalInput")
    w_g2w2 = nc.dram_tensor("w_g2w2", [HID, DIM], F32, kind="ExternalInput")
    w_g2b2 = nc.dram_tensor("w_g2b2", [DIM, 1], F32, kind="ExternalInput")
    w_pw1 = nc.dram_tensor("w_pw1", [DIM, HID], F32, kind="ExternalInput")
    w_pb1 = nc.dram_tensor("w_pb1", [HID, 1], F32, kind="ExternalInput")
    w_pw2 = nc.dram_tensor("w_pw2", [HID, HID], F32, kind="ExternalInput")
    w_pb2 = nc.dram_tensor("w_pb2", [HID, 1], F32, kind="ExternalInput")
    w_pw3 = nc.dram_tensor("w_pw3", [HID, 1], F32, kind="ExternalInput")
    w_pb3 = nc.dram_tensor("w_pb3", [1, 1], F32, kind="ExternalInput")
    eout = nc.dram_tensor("out", [1, 8], F32, kind="ExternalOutput")

    with tile.TileContext(nc) as tc:
        with tc.tile_pool(name="c1", bufs=1) as c1, \
             tc.tile_pool(name="gp", bufs=1) as gp, \
             tc.tile_pool(name="zp", bufs=2) as zp, \
             tc.tile_pool(name="sm", bufs=3) as sm, \
             tc.tile_pool(name="stg", bufs=2) as stgp, \
             tc.tile_pool(name="yp", bufs=2) as yp, \
             tc.tile_pool(name="dr", bufs=1, space="DRAM") as dr:
            ps_stack = tc.tile_pool(name="psA", bufs=4, space="PSUM")
            psA = ps_stack.__enter__()
            ps_stackB = tc.tile_pool(name="psB", bufs=3, space="PSUM")
            psB = ps_stackB.__enter__()
            ps_stackP = tc.tile_pool(name="psP", bufs=1, space="PSUM")
            psP = ps_stackP.__enter__()

            rel = nc.gpsimd.load_library(library_config.mlp)
            nig_regs = {}
            for b in range(NB):
                nig = int(off[(b + 1) * GPB] - off[b * GPB]) * P
                if nig not in nig_regs:
                    nig_regs[nig] = nc.gpsimd.to_reg(nig)

            def load(t, shape, tag, dtype=F32):
                s = c1.tile(shape, dtype, tag=tag)
                nc.sync.dma_start(out=s[:], in_=t[:])
                return s

            # --- layer-0 gather critical path: only the index grids + library
            idxw0 = load(e_idxw0, [P, C * 8], "idxw0", dtype=I16)

            # DRAM comm buffers for layers 1..L-1: compact bf16 AllGather of
            # (h|a_s) + expanded 256B-row gather table. Split: piece A covers
            # node rows [0, AGS*512), piece B the rest.
            NAG = 65
            # three exchange pieces: rows [0,1024) / [1024,2048) / [2048,2560)
            RR = (0, 1024, 2048, NPAD)
            ag_ins = [dr.tile([NPAD, NAG], BF16, tag=f"ag_in{l}", name=f"ag_in{l}")
                      if l else None for l in range(L)]
            ag_outs = [[dr.tile([NC * (RR[p + 1] - RR[p]), NAG], BF16,
                                tag=f"ag_out{p}_{l}", name=f"ag_out{p}_{l}",
                                addr_space="Shared")
                        for p in range(3)] if l else None for l in range(L)]
            gts = [dr.tile([NC * NPAD, P], BF16, tag=f"gt{l}", name=f"gt{l}")
                   if l else None for l in range(L)]

            # ---- issue ALL layer-0 gathers immediately (source table is the
            # external dict e_gt0; no AllGather needed)
            Gbs = {}

            def issue_gather(l, b):
                cb0 = int(off[b * GPB])
                CB = int(off[(b + 1) * GPB] - cb0)
                Gb = gp.tile([P, CB, NAG], BF16, tag=f"Gb{b}", bufs=1,
                             name=f"Gb{b}_{l}")
                src = e_gt0 if l == 0 else gts[l]
                idxs = idxw0 if l == 0 else idxw
                gi = nc.gpsimd.dma_gather(
                    out_ap=Gb[:], in_ap=src[:, 0:NAG],
                    idxs_ap=idxs[:, cb0 * 8:(cb0 + CB) * 8],
                    num_idxs=CB * P, num_idxs_reg=nig_regs[CB * P],
                    elem_size=NAG, elem_step=P, single_packet=False,
                    queue_num=GQ[b])
                add_dep_helper(gi.ins, rel.ins, False, "needs mlp lib")
                Gbs[b] = Gb

            for b in range(NB):
                issue_gather(0, b)

            # --- remaining loads (DMAs hide under the layer-0 gathers)
            iotac = load(w_iotac, [P, 1], "iotac")
            ident = load(w_ident, [P, P], "ident")
            T0 = load(w_t0, [P, NROW], "T0")
            Wconv = load(w_conv, [P, L * HID], "Wconv")
            Watt = load(w_att, [HID, L * 2], "Watt")
            idxw = load(e_idxw, [P, C * 8], "idxw", dtype=I16)
            AE = load(e_ae, [P, C, 6], "AE", dtype=BF16)
            aeloop = load(e_aeloop, [P, GRP, L], "aeloop")
            Wm1 = load(w_m1, [HID, L * HID], "Wm1")
            Wm2 = load(w_m2, [HID, L * HID], "Wm2")
            Wm3 = load(w_m3, [HID, L * DIM], "Wm3")
            B1 = load(w_b1, [HID, L], "B1")
            B2 = load(w_b2, [HID, L], "B2")
            B3 = load(w_b3, [DIM, L], "B3")
            Eps = load(w_eps, [P, L], "Eps")
            G1W1 = load(w_g1w1, [DIM, HID], "G1W1")
            G1B1 = load(w_g1b1, [HID, 1], "G1B1")
            G1W2 = load(w_g1w2, [HID, 1], "G1W2")
            G1B2 = load(w_g1b2, [1, 1], "G1B2")
            G2W1 = load(w_g2w1, [DIM, HID], "G2W1")
            G2B1 = load(w_g2b1, [HID, 1], "G2B1")
            G2W2 = load(w_g2w2, [HID, DIM], "G2W2")
            G2B2 = load(w_g2b2, [DIM, 1], "G2B2")
            PW1 = load(w_pw1, [DIM, HID], "PW1")
            PB1 = load(w_pb1, [HID, 1], "PB1")
            PW2 = load(w_pw2, [HID, HID], "PW2")
            PB2 = load(w_pb2, [HID, 1], "PB2")
            PW3 = load(w_pw3, [HID, 1], "PW3")
            PB3 = load(w_pb3, [1, 1], "PB3")
            Spool = load(e_spool, [P, GRP, 8], "Spool")
            Pcnt = load(e_pcnt, [8, 1], "Pcnt")

            identb = c1.tile([P, P], BF16, tag="identb")
            nc.vector.tensor_copy(out=identb[:], in_=ident[:])
            Wattb = c1.tile([HID, L * 2], BF16, tag="Wattb")
            nc.vector.tensor_copy(out=Wattb[:], in_=Watt[:])
            Spoolb = c1.tile([P, GRP, 8], BF16, tag="Spoolb")
            nc.vector.tensor_copy(out=Spoolb[:], in_=Spool[:])

            # float32r copies of matmul weights (DVE rounds on write, which the
            # BIR verifier requires of every fp32r-matmul producer)
            def rcopy(src, shape, tag):
                t = c1.tile(shape, F32R, tag=tag)
                nc.vector.tensor_copy(out=t[:], in_=src[:])
                return t

            T0r = rcopy(T0, [P, NROW], "T0r")
            Wconvr = rcopy(Wconv, [P, L * HID], "Wconvr")
            Wm1r = rcopy(Wm1, [HID, L * HID], "Wm1r")
            Wm2r = rcopy(Wm2, [HID, L * HID], "Wm2r")
            Wm3r = rcopy(Wm3, [HID, L * DIM], "Wm3r")
            Epsr = rcopy(Eps, [P, L], "Epsr")
            G1W1r = rcopy(G1W1, [DIM, HID], "G1W1r")
            G2W1r = rcopy(G2W1, [DIM, HID], "G2W1r")
            G1W2r = rcopy(G1W2, [HID, 1], "G1W2r")
            G2W2r = rcopy(G2W2, [HID, DIM], "G2W2r")

            ones1_128 = c1.tile([1, P], F32, tag="ones1_128")
            nc.vector.memset(ones1_128[:], 1.0)
            ones1r = c1.tile([1, P], F32R, tag="ones1r")
            nc.vector.tensor_copy(out=ones1r[:], in_=ones1_128[:])

            # x_idx broadcast to [128, NPAD] (partition-stride-0 DMA read)
            xidxb = c1.tile([P, NPAD], F32, tag="xbig")
            nc.sync.dma_start(out=xidxb[:], in_=AP(e_xidx, 0, [[0, P], [1, NPAD]]))

            outc = c1.tile([HID, NPAD], F32R, tag="outc")
            feat = c1.tile([P, NPAD], F32R, tag="feat")
            bn = c1.tile([1, NPAD], F32, tag="bn")      # running best jkn norm
            r0 = c1.tile([1, NPAD], F32R, tag="r0")     # is-better mask row
            h2T = c1.tile([DIM, NPAD], F32, tag="h2T")
            wrow = c1.tile([1, NPAD], F32, tag="wrow")
            pool = psP.tile([8, DIM + 1], F32, tag="pool")

            # ---- layer-0 node-major staging: nm0 = one-hot(x_idx) @ T0
            nm = stgp.tile([P, GRP, NROW], BF16, tag="nm", bufs=2, name="nm0")
            for s in range(5):
                sl = slice(s * 512, (s + 1) * 512)
                ohx = stgp.tile([P, 512], F32R, tag="stg", name="ohx")
                nc.vector.tensor_scalar(out=ohx[:], in0=xidxb[:, sl],
                                        scalar1=iotac[:], scalar2=None,
                                        op0=OP.is_equal)
                for t in range(4):
                    g = s * 4 + t
                    pnm = psB.tile([P, NROW], F32, tag="psB", name="pnm")
                    nc.tensor.matmul(out=pnm[:], lhsT=ohx[:, t * P:(t + 1) * P],
                                     rhs=T0r[:], start=True, stop=True)
                    nc.scalar.activation(nm[:, g, :], pnm[:], ACTF.Copy)

            def stage_slice(l, s, x_src, nm_t):
                """conv h | a_s | a_d for slice s of layer l -> node-major nm_t."""
                sl = slice(s * 512, (s + 1) * 512)
                ph = psB.tile([HID, 512], F32, tag="psB")
                nc.tensor.matmul(out=ph[:], lhsT=Wconvr[:, l * HID:(l + 1) * HID],
                                 rhs=x_src[:, sl], start=True, stop=True)
                stg = stgp.tile([NROW, 512], BF16, tag="stg")
                nc.scalar.activation(stg[0:HID, :], ph[:], ACTF.Copy)
                pa = psB.tile([2, 512], F32, tag="psB")
                nc.tensor.matmul(out=pa[:], lhsT=Wattb[:, l * 2:(l + 1) * 2],
                                 rhs=stg[0:HID, :], start=True, stop=True)
                nc.scalar.activation(stg[HID:HID + 2, :], pa[:], ACTF.Copy)
                for t in range(4):
                    g = s * 4 + t
                    ptr = psA.tile([P, NROW], BF16, tag="psA")
                    nc.tensor.transpose(out=ptr[:], in_=stg[:, t * 128:(t + 1) * 128],
                                        identity=identb[:NROW, :NROW])
                    if t % 2 == 0:
                        nc.vector.tensor_copy(out=nm_t[:, g, :], in_=ptr[:])
                    else:
                        nc.scalar.activation(nm_t[:, g, :], ptr[:], ACTF.Copy)
                # ship this slice's 4 groups to the comm buffer
                ago = ag_ins[l][:].rearrange("(g p) c -> p g c", p=P)
                nc.sync.dma_start(out=ago[:, s * 4:(s + 1) * 4, :],
                                  in_=nm_t[:, s * 4:(s + 1) * 4, 0:65])

            def issue_ag(l, p):
                nc.gpsimd.collective_compute(
                    "AllGather", OP.bypass, replica_groups=[list(range(NC))],
                    ins=[ag_ins[l][RR[p]:RR[p + 1], :]], outs=[ag_outs[l][p][:]])

            def issue_copy(l, p):
                rows = RR[p + 1] - RR[p]
                gv = gts[l][:].rearrange("(r n) c -> r n c", n=NPAD)
                av = ag_outs[l][p][:].rearrange("(r n) c -> r n c", n=rows)
                nc.sync.dma_start(out=gv[:, RR[p]:RR[p + 1], 0:65], in_=av[:])

            def jkn_slice(l, s, xl):
                sl = slice(s * 512, (s + 1) * 512)
                sq = sm.tile([P, 512], F32R, tag="sq", name="sq", bufs=2)
                nc.scalar.activation(sq[:], _f(xl[:, sl]), ACTF.Square)
                pml = psB.tile([1, 512], F32, tag="psB", name="pml")
                nc.tensor.matmul(out=pml[:], lhsT=Epsr[:, l:l + 1], rhs=sq[:],
                                 start=True, stop=True)
                if l == 0:
                    nc.vector.tensor_copy(out=bn[0:1, sl], in_=pml[:])
                    nc.vector.tensor_copy(out=feat[:, sl], in_=_f(xl[:, sl]))
                else:
                    nc.vector.tensor_tensor(out=r0[0:1, sl], in0=pml[:],
                                            in1=bn[0:1, sl], op=OP.is_gt)
                    nc.vector.tensor_tensor(out=bn[0:1, sl], in0=bn[0:1, sl],
                                            in1=pml[:], op=OP.max)
                    pm = psB.tile([P, 512], F32, tag="psB", name="pm")
                    nc.tensor.matmul(out=pm[:], lhsT=ones1r[:], rhs=r0[0:1, sl],
                                     start=True, stop=True)
                    ft = sm.tile([P, 512], F32, tag="ft", name="ft", bufs=2)
                    nc.vector.tensor_tensor(out=ft[:], in0=_f(xl[:, sl]),
                                            in1=_f(feat[:, sl]), op=OP.subtract)
                    nc.vector.tensor_tensor(out=ft[:], in0=ft[:], in1=pm[:],
                                            op=OP.mult)
                    nc.vector.tensor_tensor(out=feat[:, sl], in0=_f(feat[:, sl]),
                                            in1=ft[:], op=OP.add)

            def pool_slice(s):
                """Graph-pooling feature matmuls for node slice s (from feat)."""
                sl = slice(s * 512, (s + 1) * 512)
                pa1 = psB.tile([HID, 512], F32, tag="psB")
                nc.tensor.matmul(out=pa1[:], lhsT=G1W1r[:], rhs=feat[:, sl],
                                 start=True, stop=True)
                r1 = yp.tile([HID, 512], F32R, tag="y", name="r1", bufs=3)
                nc.scalar.activation(r1[:], pa1[:], ACTF.Relu, bias=G1B1[:])
                ph1 = psB.tile([1, 512], F32, tag="psB", name="ph1")
                nc.tensor.matmul(out=ph1[:], lhsT=G1W2r[:], rhs=r1[:],
                                 start=True, stop=True)
                nc.scalar.activation(wrow[0:1, sl], ph1[:], ACTF.Exp, bias=G1B2[:])
                pa2 = psB.tile([HID, 512], F32, tag="psB")
                nc.tensor.matmul(out=pa2[:], lhsT=G2W1r[:], rhs=feat[:, sl],
                                 start=True, stop=True)
                r2 = yp.tile([HID, 512], F32R, tag="y", name="r2", bufs=3)
                nc.scalar.activation(r2[:], pa2[:], ACTF.Relu, bias=G2B1[:])
                ph2 = psB.tile([DIM, 512], F32, tag="psB")
                nc.tensor.matmul(out=ph2[:], lhsT=G2W2r[:], rhs=r2[:],
                                 start=True, stop=True)
                nc.scalar.activation(h2T[:, sl], ph2[:], ACTF.Identity, bias=G2B2[:])

            x_cur = None
            pending_jkn = None
            # =================== layers ===================
            for l in range(L):
                if pending_jkn is not None:
                    pending_jkn()
                    pending_jkn = None
                # ---- self-loop weights, node-major [128, GRP]
                wloop = sm.tile([P, GRP], F32, tag="wloop")
                zt = sm.tile([P, GRP], F32, tag="zt")
                nc.vector.tensor_tensor(out=zt[:], in0=nm[:, :, 64], in1=nm[:, :, 65],
                                        op=OP.add)
                nc.vector.tensor_tensor(out=zt[:], in0=zt[:], in1=aeloop[:, :, l],
                                        op=OP.add)
                t2 = sm.tile([P, GRP], F32, tag="zt2")
                nc.scalar.activation(t2[:], zt[:], ACTF.Prelu, alpha=LRELU)
                nc.scalar.activation(wloop[:], t2[:], ACTF.Exp)
                smsg_all = sm.tile([P, GRP, 64], BF16, tag="smsg_all", bufs=1)
                nc.vector.tensor_tensor(out=smsg_all[:], in0=nm[:, :, 0:64],
                                        in1=_bc(wloop[:], 2, 64), op=OP.mult)
                # ---- per-group edge processing for one gather batch
                def edge_batch(b, l=l, nm=nm, wloop=wloop, smsg_all=smsg_all):
                    cb0 = int(off[b * GPB])
                    Gb = Gbs[b]
                    for gg in range(GPB):
                        g = b * GPB + gg
                        Mg = Ms[g]
                        gs = int(off[g])
                        cb = gs - cb0
                        # logits -> normalized weights (ACT-heavy chain)
                        z = zp.tile([P, MMAX], F32, tag="z")
                        nc.vector.scalar_tensor_tensor(
                            out=z[:, 0:Mg], in0=Gb[:, cb:cb + Mg, 64],
                            scalar=nm[:, g, 65:66], in1=AE[:, gs:gs + Mg, l],
                            op0=OP.add, op1=OP.add)
                        zl = zp.tile([P, MMAX], F32, tag="t0")
                        nc.scalar.activation(zl[:, 0:Mg], z[:, 0:Mg], ACTF.Prelu,
                                             alpha=LRELU)
                        wb = zp.tile([P, MMAX], BF16, tag="wb")
                        dn0 = sm.tile([P, 1], F32, tag="dn", bufs=2)
                        nc.scalar.activation(wb[:, 0:Mg], zl[:, 0:Mg], ACTF.Exp,
                                             accum_out=dn0[:])
                        dn = sm.tile([P, 1], F32, tag="dnf", bufs=2)
                        nc.vector.tensor_tensor(out=dn[:], in0=dn0[:],
                                                in1=wloop[:, g:g + 1], op=OP.add)
                        rec = sm.tile([P, 1], F32, tag="rec")
                        nc.vector.reciprocal(out=rec[:], in_=dn[:])
                        wb2 = zp.tile([P, MMAX, 2], BF16, tag="wb2")
                        nc.scalar.activation(wb2[:, 0:Mg, :],
                                             _bc(wb[:, 0:Mg], 1, 2), ACTF.Copy)
                        gsl = Gb[:, cb:cb + Mg, 0:64]
                        gv = AP(gsl.tensor, gsl.offset,
                                [list(gsl.ap[0]), [65, Mg], [2, 32], [1, 2]])
                        w2 = wb2[:, 0:Mg, :]
                        wv = AP(w2.tensor, w2.offset,
                                [list(w2.ap[0]), [2, Mg], [0, 32], [1, 2]])
                        nc.vector.tensor_tensor(out=gv, in0=gv, in1=wv, op=OP.mult)
                        # scatter-accumulate (node-major)
                        pg = psA.tile([P, 64], F32, tag="psA")
                        for k in range(Mg):
                            nc.tensor.matmul(out=pg[:], lhsT=identb[:],
                                             rhs=Gb[:, cb + k, 0:64],
                                             start=(k == 0), stop=False)
                        nc.tensor.matmul(out=pg[:], lhsT=identb[:],
                                         rhs=smsg_all[:, g, :],
                                         start=False, stop=True)
                        # normalize on ACT (PSUM read) + transpose to feature-major
                        onm = sm.tile([P, 64], F32, tag="onm")
                        nc.scalar.activation(onm[:], pg[:, 0:64], ACTF.Copy,
                                             scale=rec[:])
                        ptr2 = psA.tile([64, P], F32, tag="psA")
                        nc.tensor.transpose(out=ptr2[:], in_=onm[:], identity=ident[:])
                        nc.vector.tensor_copy(out=outc[:, g * P:(g + 1) * P],
                                              in_=ptr2[:])

                # ---- MLP (feature-major), fused with next-layer staging per slice
                x_new = c1.tile([P, NPAD], F32R, tag=f"xl{l % 2}", name=f"x{l}")
                if l < L - 1:
                    nm_next = stgp.tile([P, GRP, NROW], BF16, tag="nm", bufs=2,
                                        name=f"nm{l + 1}")
                else:
                    nm_next = None

                def mlp_slice(s, l=l, x_new=x_new, nm_next=nm_next):
                    sl = slice(s * 512, (s + 1) * 512)
                    p1 = psB.tile([HID, 512], F32, tag="psB")
                    nc.tensor.matmul(out=p1[:], lhsT=Wm1r[:, l * HID:(l + 1) * HID],
                                     rhs=outc[:, sl], start=True, stop=True)
                    y1 = yp.tile([HID, 512], F32R, tag="y", name="y1", bufs=3)
                    nc.scalar.activation(y1[:], p1[:], ACTF.Relu, bias=B1[:, l:l + 1])
                    p2 = psB.tile([HID, 512], F32, tag="psB")
                    nc.tensor.matmul(out=p2[:], lhsT=Wm2r[:, l * HID:(l + 1) * HID],
                                     rhs=y1[:], start=True, stop=True)
                    y2 = yp.tile([HID, 512], F32R, tag="y", name="y2", bufs=3)
                    nc.scalar.activation(y2[:], p2[:], ACTF.Relu, bias=B2[:, l:l + 1])
                    p3 = psB.tile([P, 512], F32, tag="psB")
                    nc.tensor.matmul(out=p3[:], lhsT=Wm3r[:, l * DIM:(l + 1) * DIM],
                                     rhs=y2[:], start=True, stop=True)
                    nc.scalar.activation(x_new[:, sl], p3[:], ACTF.Identity,
                                         bias=B3[:, l:l + 1])
                    if nm_next is not None:
                        stage_slice(l + 1, s, x_new, nm_next)
                    else:
                        # last layer: fold JKN + pooling feature matmuls in here
                        jkn_slice(l, s, x_new)
                        pool_slice(s)

                # interleave: MLP slice s fires right after its aligned batch.
                # Exchange piece 0 (node rows 0-1023 = slices 0-1) only needs
                # wave-1 data, so its wire time hides under the gather stream;
                # pieces 1 and 2 cover slices 2-3 / 4. Copies are issued apart
                # from the collectives so they can't head-of-line-block ships.
                edge_batch(0)
                edge_batch(1)
                mlp_slice(0)
                edge_batch(2)
                mlp_slice(1)
                if l < L - 1:
                    issue_ag(l + 1, 0)
                edge_batch(3)
                mlp_slice(2)
                if l < L - 1:
                    issue_copy(l + 1, 0)
                edge_batch(4)
                mlp_slice(3)
                if l < L - 1:
                    issue_ag(l + 1, 1)
                mlp_slice(4)
                if l < L - 1:
                    issue_ag(l + 1, 2)
                    issue_copy(l + 1, 1)
                    issue_copy(l + 1, 2)
                    for b in range(NB):
                        issue_gather(l + 1, b)
                x_cur = x_new
                nm = nm_next

                if l < L - 1:
                    def _jkn_update(xl=x_new, l=l):
                        for s in range(5):
                            jkn_slice(l, s, xl)
                    pending_jkn = _jkn_update

            # =================== pooling (node-major, permutation-proof) ===================
            for g in range(GRP):
                cs = slice(g * P, (g + 1) * P)
                # node-major exp weights and h2 for this 128-node chunk
                pwn = psA.tile([P, 1], F32, tag="psA", name="pwn")
                nc.tensor.transpose(out=pwn[:], in_=wrow[0:1, cs], identity=ident[0:1, 0:1])
                wn = sm.tile([P, 1], F32, tag="wn", bufs=2)
                nc.vector.tensor_copy(out=wn[:], in_=pwn[:])
                ph2n = psA.tile([P, DIM], F32, tag="psA", name="ph2n")
                nc.tensor.transpose(out=ph2n[:], in_=h2T[:, cs], identity=ident[:])
                whn = sm.tile([P, DIM + 1], BF16, tag="whn", bufs=2)
                nc.vector.tensor_scalar(out=whn[:, 0:DIM], in0=ph2n[:], scalar1=wn[:],
                                        scalar2=None, op0=OP.mult)
                nc.vector.tensor_copy(out=whn[:, DIM:DIM + 1], in_=wn[:])
                nc.tensor.matmul(out=pool[:], lhsT=Spoolb[:, g, :], rhs=whn[:],
                                 start=(g == 0), stop=(g == GRP - 1))

            pool_sb = sm.tile([8, DIM + 1], F32, tag="pool_sb")
            nc.vector.tensor_copy(out=pool_sb[:], in_=pool[:])
            den = sm.tile([8, 1], F32, tag="den")
            nc.vector.tensor_tensor(out=den[:], in0=pool_sb[:, DIM:DIM + 1],
                                    in1=Pcnt[:], op=OP.mult)
            rcp = sm.tile([8, 1], F32, tag="rcp")
            nc.vector.reciprocal(out=rcp[:], in_=den[:])
            nc.vector.tensor_scalar(out=pool_sb[:, 0:DIM], in0=pool_sb[:, 0:DIM],
                                    scalar1=rcp[:], scalar2=None, op0=OP.mult)
            phg = psA.tile([DIM, 8], F32, tag="psA", name="phg")
            nc.tensor.transpose(out=phg[:], in_=pool_sb[:, 0:DIM], identity=ident[:8, :8])
            hg = c1.tile([DIM, 8], F32, tag="hg")
            nc.vector.tensor_copy(out=hg[:], in_=phg[:])

            pp1 = psB.tile([HID, 8], F32, tag="psB")
            nc.tensor.matmul(out=pp1[:], lhsT=PW1[:], rhs=hg[:], start=True, stop=True)
            rp1 = sm.tile([HID, 8], F32, tag="rp1")
            nc.scalar.activation(rp1[:], pp1[:], ACTF.Relu, bias=PB1[:])
            pp2 = psB.tile([HID, 8], F32, tag="psB")
            nc.tensor.matmul(out=pp2[:], lhsT=PW2[:], rhs=rp1[:], start=True, stop=True)
            rp2 = sm.tile([HID, 8], F32, tag="rp2")
            nc.scalar.activation(rp2[:], pp2[:], ACTF.Relu, bias=PB2[:])
            pp3 = psB.tile([1, 8], F32, tag="psB")
            nc.tensor.matmul(out=pp3[:], lhsT=PW3[:], rhs=rp2[:], start=True, stop=True)
            ores = sm.tile([1, 8], F32, tag="ores")
            nc.vector.tensor_scalar(out=ores[:], in0=pp3[:], scalar1=PB3[:],
                                    scalar2=None, op0=OP.add)
            nc.sync.dma_start(out=eout[:], in_=ores[:])
            ps_stackP.__exit__(None, None, None)
            ps_stackB.__exit__(None, None, None)
            ps_stack.__exit__(None, None, None)

    lower_extended_insts(nc)
    return nc


def _prep_host(inputs):
    src = np.asarray(inputs['edge_index'][0]).astype(np.int64)
    dst = np.asarray(inputs['edge_index'][1]).astype(np.int64)
    attr = np.asarray(inputs['edge_attr_idx']).astype(np.int64)
    x_idx = np.asarray(inputs['x_idx']).astype(np.int64)
    batch = np.asarray(inputs['batch']).astype(np.int64)
    emb = np.asarray(inputs['emb']).astype(np.float32)

    conv_W = np.asarray(inputs['conv_W'], np.float32)
    conv_We = np.asarray(inputs['conv_We'], np.float32)
    att_s = np.asarray(inputs['conv_att_src'], np.float32)
    att_d = np.asarray(inputs['conv_att_dst'], np.float32)
    att_e = np.asarray(inputs['conv_att_edge'], np.float32)
    V = np.stack([conv_We[l] @ att_e[l] for l in range(L)], 1)    # [128, 6]
    t_all = (emb @ V).astype(np.float32)                          # [128, 6]

    owner = dst // NPC

    # per-core edge lists + in-degree-sorted permutations
    per_core = []
    perms = []     # perm[c][slot] = original local node (len NPC for slots < NPC)
    invs = np.zeros((NC, NPC), np.int64)
    degs = []
    for c in range(NC):
        m = np.where(owner == c)[0]
        dl = (dst[m] - c * NPC).astype(np.int64)
        counts = np.bincount(dl, minlength=NPC)
        order = np.argsort(-counts, kind='stable')  # degree desc
        inv = np.zeros(NPC, np.int64)
        inv[order] = np.arange(NPC)
        perms.append(order)
        invs[c] = inv
        degs.append(counts)
        per_core.append((m, dl, counts))

    # common per-group chunk counts (max across cores), in degree-sorted order
    Ms_sorted = np.zeros(GRP, np.int64)
    for c in range(NC):
        cs = degs[c][perms[c]]
        cs_pad = np.zeros(NPAD, np.int64)
        cs_pad[:NPC] = cs
        Ms_sorted = np.maximum(Ms_sorted, cs_pad.reshape(GRP, P).max(1))

    # pack quantile groups into NB batches with DESCENDING batch sums so the
    # last batch (whose consumers gate next-layer staging) is smallest
    GPB_ = GRP // NB
    Ctot = int(Ms_sorted.sum())
    frac = np.array([0.24, 0.22, 0.20, 0.18, 0.16])
    targets = frac * Ctot
    order_q = sorted(range(GRP), key=lambda i: -int(Ms_sorted[i]))
    batches = [[] for _ in range(NB)]
    bsums = [0.0] * NB
    for qg in order_q:
        cands = [j for j in range(NB) if len(batches[j]) < GPB_]
        i = min(cands, key=lambda j: (bsums[j] + Ms_sorted[qg]) / targets[j])
        batches[i].append(qg)
        bsums[i] += int(Ms_sorted[qg])
    border = sorted(range(NB), key=lambda j: -bsums[j])
    order_groups = [qg for j in border for qg in batches[j]]
    Ms = [int(Ms_sorted[qg]) for qg in order_groups]
    off = np.zeros(GRP + 1, np.int64)
    np.cumsum(Ms, out=off[1:])
    C = int(off[GRP])

    # slot permutation: slot j*128+p  <- degree-rank order_groups[j]*128+p
    slot_of_rank = np.zeros(NPAD, np.int64)   # rank -> slot
    for j, qg in enumerate(order_groups):
        slot_of_rank[qg * P:(qg + 1) * P] = np.arange(j * P, (j + 1) * P)

    # node_at_slot[c][slot] = original local node id (or -1 for pads)
    node_at_slot = np.full((NC, NPAD), -1, np.int64)
    for c in range(NC):
        ranks = np.arange(NPC)
        node_at_slot[c, slot_of_rank[ranks]] = perms[c]

    # global padded slot id of each source node (for gather indices)
    src_slot = np.zeros(N, np.int64)
    for c in range(NC):
        loc = np.arange(NPC)
        src_slot[c * NPC + loc] = c * NPAD + slot_of_rank[invs[c][loc]]

    def wrap16(fl):
        w = np.zeros((P, fl.shape[0] // 16), np.int16)
        for r in range(16):
            w[r::16, :] = fl[r::16].reshape(1, -1)
        return w

    import ml_dtypes
    cores = []
    for c in range(NC):
        m, dl, counts = per_core[c]
        # edges sorted by dst for segment extraction
        order_e = np.argsort(dl, kind='stable')
        eidx = m[order_e]
        dls = dl[order_e]
        starts = np.zeros(NPC + 1, np.int64)
        np.cumsum(np.bincount(dls, minlength=NPC), out=starts[1:])

        idxflat = np.zeros(C * P, np.int64)
        idxflat0 = np.zeros(C * P, np.int64)
        ae = np.full((P, C, 6), -1e9, np.float32)
        for slot_g in range(GRP):
            base = int(off[slot_g])
            Mg = Ms[slot_g]
            for p in range(P):
                slot = slot_g * P + p
                n = node_at_slot[c, slot]
                if n < 0:
                    continue
                s0, cnt = starts[n], counts[n]
                assert cnt <= Mg, (c, slot_g, p, cnt, Mg)
                es = eidx[s0:s0 + cnt]
                for k in range(cnt):
                    ch = base + k
                    idxflat[ch * P + p] = src_slot[src[es[k]]]
                    idxflat0[ch * P + p] = x_idx[src[es[k]]]
                    ae[p, ch, 0:6] = t_all[attr[es[k]]]
        idxw = wrap16(idxflat.astype(np.int16))
        idxw0 = wrap16(idxflat0.astype(np.int16))
        # per-node loop attr, permuted node-major
        ae_sum = np.zeros((NPC, L), np.float32)
        np.add.at(ae_sum, dls, t_all[attr[eidx]])
        ael = ae_sum / np.maximum(counts.astype(np.float32), 1.0)[:, None]
        ael_slot = np.zeros((NPAD, L), np.float32)
        valid = node_at_slot[c] >= 0
        ael_slot[valid] = ael[node_at_slot[c][valid]]
        aeloop = ael_slot.reshape(GRP, P, L).transpose(1, 0, 2).copy()
        # pooling one-hot [P, GRP, 8] and x_idx per slot
        spool = np.zeros((NPAD, 8), np.float32)
        gids = batch[c * NPC + node_at_slot[c][valid]] - c * 8
        spool[np.where(valid)[0], gids] = 1.0
        spool = spool.reshape(GRP, P, 8).transpose(1, 0, 2).copy()
        xi = np.full(NPAD, -1.0, np.float32)
        xi[valid] = x_idx[c * NPC + node_at_slot[c][valid]].astype(np.float32)
        cnts = np.bincount(batch[c * NPC:(c + 1) * NPC] - c * 8,
                           minlength=8).astype(np.float32)
        cores.append(dict(e_idxw=idxw, e_idxw0=idxw0,
                          e_ae=ae.astype(ml_dtypes.bfloat16),
                          e_aeloop=aeloop, e_spool=spool,
                          e_xidx=xi, e_pcnt=cnts.reshape(8, 1)))

    # ---- shared weights
    conv_b = np.asarray(inputs['conv_b'], np.float32)
    m1 = np.asarray(inputs['mlp_W1'], np.float32)
    m2 = np.asarray(inputs['mlp_W2'], np.float32)
    m3 = np.asarray(inputs['mlp_W3'], np.float32)
    b1 = np.asarray(inputs['mlp_b1'], np.float32)
    b2 = np.asarray(inputs['mlp_b2'], np.float32)
    b3 = np.asarray(inputs['mlp_b3'], np.float32)
    b1_eff = np.stack([conv_b[l] @ m1[l] + b1[l] for l in range(L)], 1)

    # layer-0 dictionary: h0 = emb@W0, a_s0/a_d0 per embedding id
    h0 = emb @ conv_W[0]                      # [128, 64]
    a_s0 = h0 @ att_s[0]
    a_d0 = h0 @ att_d[0]
    gt0 = np.zeros((P, P), np.float32)
    gt0[:, 0:HID] = h0
    gt0[:, HID] = a_s0
    t0 = np.zeros((P, NROW), np.float32)
    t0[:, 0:HID] = h0
    t0[:, HID] = a_s0
    t0[:, HID + 1] = a_d0

    shared = dict(
        w_iotac=np.arange(P, dtype=np.float32).reshape(P, 1),
        w_ident=np.eye(P, dtype=np.float32),
        e_gt0=gt0.astype(ml_dtypes.bfloat16),
        w_t0=t0,
        w_conv=np.concatenate([conv_W[l] for l in range(L)], 1),
        w_att=np.concatenate([np.stack([att_s[l], att_d[l]], 1) for l in range(L)], 1),
        w_m1=np.concatenate([m1[l] for l in range(L)], 1),
        w_m2=np.concatenate([m2[l] for l in range(L)], 1),
        w_m3=np.concatenate([m3[l] for l in range(L)], 1),
        w_b1=b1_eff,
        w_b2=b2.T.copy(),
        w_b3=b3.T.copy(),
        w_eps=np.broadcast_to((1.0 - np.arange(L, dtype=np.float32) * 1e-7)[None, :],
                              (P, L)).copy(),
        w_g1w1=np.asarray(inputs['g1_W1'], np.float32),
        w_g1b1=np.asarray(inputs['g1_b1'], np.float32).reshape(HID, 1),
        w_g1w2=np.asarray(inputs['g1_W2'], np.float32),
        w_g1b2=np.asarray(inputs['g1_b2'], np.float32).reshape(1, 1),
        w_g2w1=np.asarray(inputs['g2_W1'], np.float32),
        w_g2b1=np.asarray(inputs['g2_b1'], np.float32).reshape(HID, 1),
        w_g2w2=np.asarray(inputs['g2_W2'], np.float32),
        w_g2b2=np.asarray(inputs['g2_b2'], np.float32).reshape(DIM, 1),
        w_pw1=np.asarray(inputs['p_W1'], np.float32),
        w_pb1=np.asarray(inputs['p_b1'], np.float32).reshape(HID, 1),
        w_pw2=np.asarray(inputs['p_W2'], np.float32),
        w_pb2=np.asarray(inputs['p_b2'], np.float32).reshape(HID, 1),
        w_pw3=np.asarray(inputs['p_W3'], np.float32),
        w_pb3=np.asarray(inputs['p_b3'], np.float32).reshape(1, 1),
    )

    in_maps = []
    for c in range(NC):
        mm = dict(shared)
        mm.update(cores[c])
        in_maps.append(mm)
    return tuple(Ms), in_maps


def kernel(**inputs):
    Ms, in_maps = _prep_host(inputs)
    if Ms not in _cache:
        _cache[Ms] = _build(Ms)
    nc = _cache[Ms]
    res = run_bass_kernel_spmd(nc, in_maps, core_ids=list(range(NC)))
    out = np.concatenate([np.asarray(res.results[c]['out']).reshape(8)
                          for c in range(NC)])
    return out.astype(np.float32)


if __name__ == "__main__":
    import jax
    sys.path.insert(0, '/root/problem')
    import reference as R
    with jax.default_device(jax.devices('cpu')[0]):
        inp = R.setup_inputs()
        exp = np.asarray(R.reference(**inp))
    inp = {k: np.asarray(v) for k, v in inp.items()}
    act = kernel(**inp)
    rel = np.linalg.norm(act - exp) / np.linalg.norm(exp)
    print("Relative error:", rel)
